# revision 13
# baseline (speedup 1.0000x reference)
"""Trainium2 Bass kernel for nn_AttnBlock (sparse GQA attention block).

Sharding: 8 cores = batch(2) x head-group(4). Each core handles one batch's
sequence with 4 q-heads + their shared kv-head (GQA group), computes its
partial output projection; host sums the 4 group partials per batch and adds
the residual.

Device kernel (per core, SPMD): x^T (bf16, host-pretransposed) -> token rms
stats (square + ones-matmul + rsqrt via ln/exp) -> qkv projection
(token-major psum, rms scale folded into the V evict; q/k rms scale folded
ahead of rope, which is linear) -> rope -> PE transposes to feature-major
q/k -> block-sparse attention over the host-derived k-tile structure
(scores computed transposed [k,q]; additive masks added via identity
matmuls; exp without max subtraction, safe because |s| <= 8 for rms-normed
q,k; denominators via all-ones matmul broadcast + reciprocal_approx_fast)
-> PV into head-paired partition layout -> output projection -> partial
out^T (bf16) to DRAM.
"""

import sys
from contextlib import ExitStack

for _p in ("/opt/trn_rl_repo",):
    if _p not in sys.path:
        sys.path.insert(0, _p)

import numpy as np
import ml_dtypes

import concourse.bass as bass
import concourse.tile as tile
import concourse.mybir as mybir
from concourse.masks import make_identity

F32 = mybir.dt.float32
BF16 = mybir.dt.bfloat16
BF = ml_dtypes.bfloat16

B, L, D = 2, 2048, 1024
HEADS, KV_HEADS, DH = 16, 4, 64
WINDOW = 1024
NEG = -1e30
EPS = 1.1920929e-07
NT = L // 128          # 16 token tiles
ND = D // 128          # 8 d tiles
NG = 4                 # head groups (= cores per batch)
SCALE = 1.0 / np.sqrt(DH)


def split_multi_waits(nc):
    """This environment's walrus supports only ONE sync wait per instruction.
    Split each multi-wait instruction into single-wait NoOps inserted just
    before it (same engine; per-engine execution is in-order, so consecutive
    single waits are equivalent to one multi-wait)."""
    for func in nc.m.functions:
        for block in func.blocks:
            new_list = []
            for inst in block.instructions:
                si = inst.sync_info
                if si is not None and len(si.on_wait) > 1:
                    waits = list(si.on_wait)
                    for w in waits[:-1]:
                        new_list.append(mybir.InstNoOp(
                            name=f"waitsplit-{nc.next_id()}",
                            engine=inst.engine,
                            sync_info=mybir.SyncInfo(on_wait=[w], on_update=[]),
                            text_hint="waitsplit", bass_nofuse=True))
                    inst.sync_info = mybir.SyncInfo(
                        on_wait=[waits[-1]], on_update=list(si.on_update))
                if si is not None and len(si.on_update) > 1:
                    ups = list(inst.sync_info.on_update)
                    inst.sync_info = mybir.SyncInfo(
                        on_wait=list(inst.sync_info.on_wait), on_update=[ups[0]])
                    new_list.append(inst)
                    for u in ups[1:]:
                        new_list.append(mybir.InstNoOp(
                            name=f"updsplit-{nc.next_id()}",
                            engine=inst.engine,
                            sync_info=mybir.SyncInfo(on_wait=[], on_update=[u]),
                            text_hint="updsplit", bass_nofuse=True))
                    continue
                new_list.append(inst)
            block.instructions[:] = new_list


# ---------------------------------------------------------------- host plan

def plan_structure(reset_mask: np.ndarray):
    """Derive the union block-sparse structure and per-batch additive masks."""
    lo = np.zeros((B, L), np.int64)
    idx = np.arange(L)
    for b in range(B):
        r = np.where(np.asarray(reset_mask[b], bool), idx, 0)
        last_reset = np.maximum.accumulate(r)
        lo[b] = np.maximum(last_reset, idx - (WINDOW - 1))

    kts = []
    for qt in range(NT):
        kt_min = min(int(lo[b, 128 * qt] // 128) for b in range(B))
        kts.append(list(range(kt_min, qt + 1)))

    pairs = [(qt, kt) for qt in range(NT) for kt in kts[qt]]
    masks = np.zeros((B, len(pairs), 128, 128), np.float32)
    kk = idx[:128]
    for b in range(B):
        for i, (qt, kt) in enumerate(pairs):
            k = 128 * kt + kk[:, None]             # [128,1] global k
            q = 128 * qt + kk[None, :]             # [1,128] global q
            valid = (k >= lo[b, 128 * qt:128 * qt + 128][None, :]) & (k <= q)
            masks[b, i] = np.where(valid, 0.0, NEG)
    return kts, masks


# ------------------------------------------------------------ device build

def build_program(kts):
    pairs = [(qt, kt) for qt in range(NT) for kt in kts[qt]]
    pair_idx = {p: i for i, p in enumerate(pairs)}
    NP = len(pairs)
    NKmax = max(len(k) for k in kts)
    small_bufs = 1 if NKmax > 2 else 2

    nc = bass.Bass("TRN2", target_bir_lowering=False, debug=False, num_devices=8)
    ap_xT = nc.dram_tensor("xT", [ND, 128, L], BF16, kind="ExternalInput").ap()
    ap_wqkvT = nc.dram_tensor("wqkvT", [ND, 128, 384], BF16, kind="ExternalInput").ap()
    ap_woutP = nc.dram_tensor("woutP", [2, 128, D], BF16, kind="ExternalInput").ap()
    ap_cosF = nc.dram_tensor("cosF", [NT, 128, DH], BF16, kind="ExternalInput").ap()
    ap_sinF2 = nc.dram_tensor("sinF2", [NT, 128, DH], BF16, kind="ExternalInput").ap()
    ap_masks = nc.dram_tensor("masks", [NP, 128, 128], BF16, kind="ExternalInput").ap()
    ap_outT = nc.dram_tensor("outT", [D, L], BF16, kind="ExternalOutput").ap()

    with tile.TileContext(nc) as tc, ExitStack() as ctx:
        csts = ctx.enter_context(tc.tile_pool(name="consts", bufs=1))
        big = ctx.enter_context(tc.tile_pool(name="big", bufs=1))
        dramp = ctx.enter_context(tc.tile_pool(name="dram", bufs=1, space="DRAM"))

        ident = csts.tile([128, 128], BF16, tag="ident")
        allones = csts.tile([128, 128], BF16, tag="allones")
        make_identity(nc, ident)
        nc.gpsimd.memset(allones, 1.0)

        wqkv_sb = csts.tile([128, ND, 384], BF16, tag="wqkv")
        nc.sync.dma_start(out=wqkv_sb, in_=ap_wqkvT.rearrange("n p f -> p n f"))
        wout_sb = csts.tile([128, 2, D], BF16, tag="wout")
        nc.sync.dma_start(out=wout_sb, in_=ap_woutP.rearrange("n p f -> p n f"))
        cos_sb = csts.tile([128, NT, DH], BF16, tag="cos")
        nc.sync.dma_start(out=cos_sb, in_=ap_cosF.rearrange("n p f -> p n f"))
        sin_sb = csts.tile([128, NT, DH], BF16, tag="sin")
        nc.sync.dma_start(out=sin_sb, in_=ap_sinF2.rearrange("n p f -> p n f"))
        mask_sb = csts.tile([128, NP, 128], BF16, tag="mask")
        nc.sync.dma_start(out=mask_sb, in_=ap_masks.rearrange("n p f -> p n f"))

        xT_sb = big.tile([128, ND, L], BF16, tag="xT")
        for dj in range(ND):
            nc.sync.dma_start(out=xT_sb[:, dj, :], in_=ap_xT[dj])

        qk_raw = big.tile([128, NT, 5, DH], BF16, tag="qk_raw")
        qk_rot = big.tile([128, NT, 5, DH], BF16, tag="qk_rot")
        v_sb = big.tile([128, NT, DH], BF16, tag="v")
        qTp0 = big.tile([128, NT, 128], BF16, tag="qTp0")   # heads 0,1 on partition halves
        qTp1 = big.tile([128, NT, 128], BF16, tag="qTp1")   # heads 2,3
        kvT = big.tile([128, NT, 128], BF16, tag="kvT")     # kT in BOTH partition halves
        yTn2 = big.tile([128, 2, NT, 128], BF16, tag="yTn2")
        s_cols = big.tile([128, NT], F32, tag="s_cols")
        ms_qk = big.tile([128, NT, 5], F32, tag="ms_qk")
        s_qk = big.tile([128, NT, 5], F32, tag="s_qk")
        ms_sb = big.tile([1, L], F32, tag="ms_sb")
        s_ms = big.tile([128, NT], F32, tag="s_ms")
        scratch_dram = dramp.tile([L], F32)

        def sb_ap(t, offset_elems, dims):
            return bass.AP(tensor=t.tensor, offset=t.offset + offset_elems,
                           ap=[t.ap[0]] + dims)

        # ---- phase 1+2: rms stats + qkv projection + rope + transpose ----
        with tc.tile_pool(name="x2p", bufs=2) as x2p, \
             tc.tile_pool(name="msps", bufs=1, space="PSUM") as msps, \
             tc.tile_pool(name="qkvps", bufs=2, space="PSUM") as qkvps, \
             tc.tile_pool(name="trps", bufs=2, space="PSUM") as trps, \
             tc.tile_pool(name="sqp", bufs=2) as sqp, \
             tc.tile_pool(name="qsc", bufs=2) as qsc:
            ms_ps = msps.tile([1, 4, 512], F32)
            for dj in range(ND):
                x2 = x2p.tile([128, L], BF16, tag="x2")
                eng = nc.vector if dj % 2 == 0 else nc.gpsimd
                eng.tensor_mul(x2, xT_sb[:, dj, :], xT_sb[:, dj, :])
                for c in range(4):
                    nc.tensor.matmul(
                        ms_ps[:, c, :], allones[:, 0:1], x2[:, 512 * c:512 * c + 512],
                        start=(dj == 0), stop=(dj == ND - 1))
            nc.scalar.activation(out=ms_sb.rearrange("p (a b) -> p a b", a=4),
                                 in_=ms_ps,
                                 func=mybir.ActivationFunctionType.Copy)
            nc.sync.dma_start(out=scratch_dram, in_=ms_sb)
            nc.sync.dma_start(out=s_ms,
                              in_=scratch_dram.rearrange("(c p) -> p c", p=128))
            # s = rsqrt(ms/D + eps) = exp(-0.5*ln(ms/D + eps))
            nc.vector.tensor_scalar(out=s_ms, in0=s_ms, scalar1=1.0 / D, scalar2=EPS,
                                    op0=mybir.AluOpType.mult, op1=mybir.AluOpType.add)
            nc.scalar.activation(out=s_ms, in_=s_ms, func=mybir.ActivationFunctionType.Ln)
            nc.scalar.activation(out=s_cols, in_=s_ms,
                                 func=mybir.ActivationFunctionType.Exp, scale=-0.5)

            for ti in range(NT):
                qkv_ps = qkvps.tile([128, 384], F32)
                for dj in range(ND):
                    nc.tensor.matmul(
                        qkv_ps, xT_sb[:, dj, 128 * ti:128 * ti + 128],
                        wqkv_sb[:, dj, :], start=(dj == 0), stop=(dj == ND - 1))
                qk_view = qkv_ps[:, 0:320].rearrange("p (h d) -> p h d", d=DH)
                if ti % 2 == 0:
                    nc.vector.tensor_copy(qk_raw[:, ti, :, :], qk_view)
                else:
                    nc.scalar.activation(out=qk_raw[:, ti, :, :], in_=qk_view,
                                         func=mybir.ActivationFunctionType.Copy)
                nc.vector.tensor_scalar_mul(v_sb[:, ti, :], qkv_ps[:, 320:384],
                                            s_cols[:, ti:ti + 1])
                sq = sqp.tile([128, 5, DH], BF16, tag="sq")
                nc.gpsimd.tensor_mul(sq, qk_raw[:, ti, :, :], qk_raw[:, ti, :, :])
                nc.vector.tensor_reduce(out=ms_qk[:, ti, :], in_=sq,
                                        axis=mybir.AxisListType.X,
                                        op=mybir.AluOpType.add)

            ms_flat = ms_qk.rearrange("p a b -> p (a b)")
            s_flat = s_qk.rearrange("p a b -> p (a b)")
            nc.vector.tensor_scalar(out=ms_flat, in0=ms_flat, scalar1=1.0 / DH,
                                    scalar2=EPS, op0=mybir.AluOpType.mult,
                                    op1=mybir.AluOpType.add)
            nc.scalar.activation(out=ms_flat, in_=ms_flat,
                                 func=mybir.ActivationFunctionType.Ln)
            nc.scalar.activation(out=s_flat, in_=ms_flat,
                                 func=mybir.ActivationFunctionType.Exp, scale=-0.5)

            for ti in range(NT):
                qks = qsc.tile([128, 5, DH], BF16, tag="qks")
                for h in range(5):
                    nc.vector.tensor_scalar_mul(qks[:, h, :], qk_raw[:, ti, h, :],
                                                s_qk[:, ti, h:h + 1])
                cos_b = sb_ap(cos_sb, ti * DH, [[0, 5], [1, DH]])
                sin_b = sb_ap(sin_sb, ti * DH, [[0, 5], [1, DH]])
                half = DH // 2
                qswap = sb_ap(qks, half, [[DH, 5], [-half, 2], [1, half]])
                ra = qsc.tile([128, 5, DH], BF16, tag="ra")
                rb = qsc.tile([128, 5, DH], BF16, tag="rb")
                nc.gpsimd.tensor_mul(ra, qks, cos_b)
                nc.gpsimd.tensor_mul(rb, qswap, sin_b)
                nc.gpsimd.tensor_add(qk_rot[:, ti, :, :], ra, rb)

                # transposes: (q0,q1) -> qTp0, (q2,q3) -> qTp1, and kT into
                # BOTH partition halves of kvT (base-partition matching for
                # even/odd-head score matmuls): a [128,64] k transpose for the
                # low half and a (q3,k) [128,128] transpose whose upper half
                # is kT for the high half.
                for tj, (dst, sl) in enumerate((
                        (qTp0[:, ti, :], qk_rot[:, ti, 0:2, :]),
                        (qTp1[:, ti, :], qk_rot[:, ti, 2:4, :]))):
                    tr = trps.tile([128, 128], BF16, tag="tr")
                    nc.tensor.transpose(tr, sl, ident)
                    if (ti * 3 + tj) % 2 == 0:
                        nc.vector.tensor_copy(dst, tr)
                    else:
                        nc.scalar.activation(out=dst, in_=tr,
                                             func=mybir.ActivationFunctionType.Copy)
                trk = trps.tile([64, 128], BF16, tag="tr")
                nc.tensor.transpose(trk, qk_rot[:, ti, 4, :], ident)
                nc.vector.tensor_copy(kvT[0:64, ti, :], trk)
                trk2 = trps.tile([128, 128], BF16, tag="tr")
                nc.tensor.transpose(trk2, qk_rot[:, ti, 3:5, :], ident)
                nc.scalar.activation(out=kvT[64:128, ti, :], in_=trk2[64:128, :],
                                     func=mybir.ActivationFunctionType.Copy)

        # ---- phase 5: attention per q tile -------------------------------
        with tc.tile_pool(name="sps", bufs=1, space="PSUM") as sps, \
             tc.tile_pool(name="denps", bufs=small_bufs, space="PSUM") as denps, \
             tc.tile_pool(name="y2ps", bufs=small_bufs, space="PSUM") as y2ps, \
             tc.tile_pool(name="ops", bufs=2, space="PSUM") as ops, \
             tc.tile_pool(name="ptp", bufs=2) as ptp, \
             tc.tile_pool(name="rp", bufs=2) as rp, \
             tc.tile_pool(name="osb", bufs=3) as osb:
            for qt in range(NT):
                nk = len(kts[qt])
                s_ps = sps.tile([128, NKmax, 4, 128], F32, tag="s_ps")
                for ki, kt in enumerate(kts[qt]):
                    for h in range(4):
                        qsl = (qTp0 if h < 2 else qTp1)
                        base = 64 * (h % 2)
                        rhs = qsl[base:base + 64, qt, :]
                        nc.tensor.matmul(s_ps[:, ki, h, :],
                                         kvT[base:base + 64, kt, :], rhs,
                                         start=True, stop=False)
                        nc.tensor.matmul(s_ps[:, ki, h, :], ident,
                                         mask_sb[:, pair_idx[(qt, kt)], :],
                                         start=False, stop=True)
                pT = ptp.tile([128, NKmax, 4, 128], BF16, tag="pT")
                nc.scalar.activation(out=pT[:, 0:nk, :, :], in_=s_ps[:, 0:nk, :, :],
                                     func=mybir.ActivationFunctionType.Exp,
                                     scale=SCALE)
                den_ps = denps.tile([128, 512], F32, tag="den")
                for ki in range(nk):
                    nc.tensor.matmul(den_ps, allones,
                                     pT[:, ki, :, :],
                                     start=(ki == 0), stop=(ki == nk - 1))
                # 1/den via exp(-ln(den)) on ACT: same table set as the
                # softmax Exp (natural_log_exp), so no table-switch cost.
                r_ln = rp.tile([128, 512], F32, tag="r_ln")
                nc.scalar.activation(out=r_ln, in_=den_ps,
                                     func=mybir.ActivationFunctionType.Ln)
                r_t = rp.tile([128, 512], F32, tag="r_t")
                nc.scalar.activation(out=r_t, in_=r_ln,
                                     func=mybir.ActivationFunctionType.Exp,
                                     scale=-1.0)
                y2_ps = y2ps.tile([128, 2, 128], F32, tag="y2")
                for h in range(4):
                    for ki, kt in enumerate(kts[qt]):
                        nc.tensor.matmul(
                            y2_ps[64 * (h % 2):64 * (h % 2) + 64, h // 2, :],
                            v_sb[:, kt, :], pT[:, ki, h, :],
                            start=(ki == 0), stop=(ki == nk - 1))
                r_even = bass.AP(tensor=r_t.tensor, offset=r_t.offset,
                                 ap=[[r_t.ap[0][0], 64], [256, 2], [1, 128]])
                r_odd = bass.AP(tensor=r_t.tensor, offset=r_t.offset + 128,
                                ap=[[r_t.ap[0][0], 64], [256, 2], [1, 128]])
                nc.vector.tensor_mul(yTn2[0:64, :, qt, :], y2_ps[0:64, :, :], r_even)
                nc.vector.tensor_mul(yTn2[64:128, :, qt, :], y2_ps[64:128, :, :], r_odd)

            # ---- phase 6: output projection ------------------------------
            for dt in range(ND):
                for c in range(4):
                    o_ps = ops.tile([128, 512], F32, tag="o_ps")
                    for pair in range(2):
                        nc.tensor.matmul(
                            o_ps,
                            wout_sb[:, pair, 128 * dt:128 * dt + 128],
                            yTn2[:, pair, 4 * c:4 * c + 4, :].rearrange(
                                "p a b -> p (a b)"),
                            start=(pair == 0), stop=(pair == 1))
                    o_sb = osb.tile([128, 512], BF16, tag="o_sb")
                    if (dt * 4 + c) % 2 == 0:
                        nc.vector.tensor_copy(o_sb, o_ps)
                    else:
                        nc.scalar.activation(out=o_sb, in_=o_ps,
                                             func=mybir.ActivationFunctionType.Copy)
                    nc.sync.dma_start(out=ap_outT[128 * dt:128 * dt + 128,
                                                  512 * c:512 * c + 512], in_=o_sb)

    return nc


# ------------------------------------------------------------- host driver

_COS_SIN = None


def _cos_sin():
    global _COS_SIN
    if _COS_SIN is None:
        half = DH // 2
        inv_freq = 1.0 / (10000.0 ** (np.arange(half, dtype=np.float32) / half))
        f = np.outer(np.arange(L, dtype=np.float32), inv_freq)
        cosF = np.concatenate([np.cos(f), np.cos(f)], -1).astype(BF).reshape(NT, 128, DH)
        sinF2 = np.concatenate([-np.sin(f), np.sin(f)], -1).astype(BF).reshape(NT, 128, DH)
        _COS_SIN = (cosF, sinF2)
    return _COS_SIN


def make_core_inputs(x, w_qkv, w_out, masks, b, g):
    xT = np.ascontiguousarray(x[b].T).astype(BF).reshape(ND, 128, L)
    wg = np.concatenate([
        w_qkv[256 * g:256 * g + 256],
        w_qkv[1024 + 64 * g:1024 + 64 * g + 64],
        w_qkv[1280 + 64 * g:1280 + 64 * g + 64]], 0)        # [384, 1024]
    wqkvT = np.ascontiguousarray(wg.T).astype(BF).reshape(ND, 128, 384)
    woutP = np.stack([
        np.ascontiguousarray(w_out[:, 256 * g + 128 * p:256 * g + 128 * p + 128].T)
        for p in range(2)]).astype(BF)                       # [2, 128, 1024]
    cosF, sinF2 = _cos_sin()
    return {
        "xT": xT, "wqkvT": wqkvT, "woutP": woutP,
        "cosF": cosF, "sinF2": sinF2,
        "masks": np.ascontiguousarray(masks[b]).astype(BF),
    }


_PROGRAM_CACHE = {}


def get_program(kts):
    key = tuple(tuple(k) for k in kts)
    if key not in _PROGRAM_CACHE:
        _PROGRAM_CACHE[key] = build_program(kts)
    return _PROGRAM_CACHE[key]


def kernel(x, w_qkv, w_out, reset_mask):
    x = np.asarray(x, np.float32)
    w_qkv = np.asarray(w_qkv, np.float32)
    w_out = np.asarray(w_out, np.float32)
    reset_mask = np.asarray(reset_mask)

    kts, masks = plan_structure(reset_mask)
    nc = get_program(kts)
    if not getattr(nc, "_waitsplit_done", False):
        split_multi_waits(nc)
        nc._waitsplit_done = True

    in_maps = [make_core_inputs(x, w_qkv, w_out, masks, b, g)
               for b in range(B) for g in range(NG)]
    from concourse import bass_utils
    res = bass_utils.run_bass_kernel_spmd(nc, in_maps, core_ids=list(range(8)))

    out = x.copy()
    core = 0
    for b in range(B):
        acc = np.zeros((D, L), np.float32)
        for g in range(NG):
            acc += res.results[core]["outT"].astype(np.float32)
            core += 1
        out[b] += acc.T
    return out
